# revision 40
# baseline (speedup 1.0000x reference)
"""Encoder kernel builder for nn_Encoder (conv stack + segment-mean) on TRN2.
See layout notes in docstring history; key contracts:
  h0_nat [128=(co16*8+dy), 64, 513]; d0_nat [128=(co32*4+dy), 64, 257]
  d1_nat [128=(co64*2+dy), 64, 129]; d2_nat [128, 65, 65] (tl pad)
  d3_nat [128, 2, 33, 33] (br pad);  u0_nat [128, 65, 65] (br pad)
  u1_nat [128=(co64*2+df), 65, 65, 2]; u2_nat [128=(co32*4+df), 65, 129, 2]
  u3_nat [128=(co16*8+df), 64, 260, 2] (X-linear idx = X+4, reflect cols)
  f_nat  [96=(co3*32+dy), 16, 512]
"""
import contextlib
import numpy as np
import ml_dtypes
import concourse.bass as bass
import concourse.tile as tile
from concourse import mybir

BF16 = mybir.dt.bfloat16
F32 = mybir.dt.float32
EPS = 1e-5
AL = mybir.AluOpType
AF = mybir.ActivationFunctionType
P = 512 * 512


def _bf(x):
    return np.ascontiguousarray(x.astype(ml_dtypes.bfloat16))


def _tc_entry(w, a, b, va, vb):
    ka = a + 1 - 2 * va
    kb = b + 1 - 2 * vb
    if 0 <= ka < 3 and 0 <= kb < 3:
        return w[:, :, ka, kb]
    return None


def pack_inputs(inp):
    out = {}
    x = np.asarray(inp['x'], np.float32)
    out['x_pad'] = _bf(np.pad(x, ((0, 0), (3, 3), (3, 3)), mode='reflect'))
    out['ids'] = _bf(np.asarray(inp['instance_map']).reshape(-1).astype(np.float32))

    # L0 direct-AP: K=96=(c,rp,s 4 x-shift slots), taps (dg,kxp); kx=4*kxp+s
    w = np.asarray(inp['w_in'], np.float32)  # [16, 3, 7, 7]
    wl = np.zeros((2, 2, 96, 128), np.float32)
    for dg in range(2):
        for kxp in range(2):
            for c in range(3):
                for rp in range(8):
                    for s in range(4):
                        kx = 4 * kxp + s
                        if kx >= 7:
                            continue
                        for dy in range(8):
                            ky = 8 * dg + rp - dy
                            if 0 <= ky < 7:
                                wl[dg, kxp, c * 32 + rp * 4 + s, dy::8] = \
                                    w[:, c, ky, kx]
    out['w_l0'] = _bf(wl)

    # D-stages: direct-AP taps [2(main/corr), 3 kx, 128 K=(ci,df), 128 M=(co,dy)]
    w = np.asarray(inp['w_d0'], np.float32)  # [32, 16, 3, 3]
    wd = np.zeros((2, 3, 128, 128), np.float32)
    for kx in range(3):
        for ci in range(16):
            for df in range(8):
                for dy in range(4):
                    ky = df - 2 * dy + 1
                    if 0 <= ky < 3:
                        wd[0, kx, ci * 8 + df, dy::4] = w[:, ci, ky, kx]
            wd[1, kx, ci * 8 + 7, 0::4] = w[:, ci, 0, kx]
    out['w_d0'] = _bf(wd)

    w = np.asarray(inp['w_d1'], np.float32)  # [64, 32, 3, 3]
    wd = np.zeros((2, 3, 128, 128), np.float32)
    for kx in range(3):
        for ci in range(32):
            for df in range(4):
                for dy in range(2):
                    ky = df - 2 * dy + 1
                    if 0 <= ky < 3:
                        wd[0, kx, ci * 4 + df, dy::2] = w[:, ci, ky, kx]
            wd[1, kx, ci * 4 + 3, 0::2] = w[:, ci, 0, kx]
    out['w_d1'] = _bf(wd)

    w = np.asarray(inp['w_d2'], np.float32)  # [128, 64, 3, 3]
    wd = np.zeros((2, 3, 128, 128), np.float32)
    for kx in range(3):
        for ci in range(64):
            for df in range(2):
                wd[0, kx, ci * 2 + df, :] = w[:, ci, df + 1, kx]
            wd[1, kx, ci * 2 + 1, :] = w[:, ci, 0, kx]
    out['w_d2'] = _bf(wd)

    w = np.asarray(inp['w_d3'], np.float32)
    wd3 = np.zeros((2, 9, 128, 128), np.float32)
    for h in range(2):
        for ky in range(3):
            for kx in range(3):
                wd3[h, ky * 3 + kx] = w[128 * h:128 * h + 128, :, ky, kx].T
    out['w_d3'] = _bf(wd3)

    w = np.asarray(inp['w_u0'], np.float32)
    wu0 = np.zeros((2, 2, 2, 2, 2, 128, 128), np.float32)
    for h in range(2):
        for a in range(2):
            for b in range(2):
                for va in range(2):
                    for vb in range(2):
                        e = _tc_entry(w[128 * h:128 * h + 128], a, b, va, vb)
                        if e is not None:
                            wu0[h, a, b, va, vb] = e
    out['w_u0'] = _bf(wu0)

    w = np.asarray(inp['w_u1'], np.float32)
    wu1 = np.zeros((2, 2, 2, 128, 128), np.float32)
    for b in range(2):
        for t in range(2):
            for hoff in range(2):
                for df in range(2):
                    a = df % 2
                    va = t - (df - a) // 2
                    if va not in (0, 1):
                        continue
                    e = _tc_entry(w, a, b, va, hoff)
                    if e is not None:
                        wu1[b, t, hoff, :, df::2] = e
    out['w_u1'] = _bf(wu1)

    # U2/U3: direct-AP tconv classes [6, 128 K=(ci,df_in), 128 M=(co,dfY)]
    w = np.asarray(inp['w_u2'], np.float32)  # [64, 32, 3, 3] torch (ci, co)
    wu = np.zeros((6, 128, 128), np.float32)
    pm = [(0, 0, 1), (1, 1, 0), (0, 1, 2), (1, 2, 1), (1, 3, 2)]
    pc = [(0, 3, 0)]
    for cls, pairs, kx in [(0, pm, 1), (1, pc, 1), (2, pm, 0), (3, pm, 2),
                           (4, pc, 0), (5, pc, 2)]:
        for (df1, dfY, ky) in pairs:
            for ci in range(64):
                wu[cls, ci * 2 + df1, dfY::4] = w[ci, :, ky, kx]
    out['w_u2'] = _bf(wu)

    w = np.asarray(inp['w_u3'], np.float32)  # [32, 16, 3, 3]
    wu = np.zeros((6, 128, 128), np.float32)
    pm = [(0, 0, 1), (1, 2, 1), (2, 4, 1), (3, 6, 1), (1, 1, 0), (0, 1, 2),
          (2, 3, 0), (1, 3, 2), (3, 5, 0), (2, 5, 2), (3, 7, 2)]
    pc = [(0, 7, 0)]
    for cls, pairs, kx in [(0, pm, 1), (1, pc, 1), (2, pm, 0), (3, pm, 2),
                           (4, pc, 0), (5, pc, 2)]:
        for (df2, dfY, ky) in pairs:
            for ci in range(32):
                wu[cls, ci * 4 + df2, dfY::8] = w[ci, :, ky, kx]
    out['w_u3'] = _bf(wu)

    # Lout: direct-AP 7x7 taps; M = (dy*4 + q), q=3 rows produce ones via bias
    w = np.asarray(inp['w_out'], np.float32)  # [3, 16, 7, 7]
    wlo = np.zeros((6, 7, 128, 128), np.float32)
    for e in range(6):
        for kx in range(7):
            for ci in range(16):
                for df in range(8):
                    for dy in range(32):
                        ky = 8 * (e - 1) + df - dy + 3
                        if 0 <= ky < 7:
                            wlo[e, kx, ci * 8 + df,
                                dy * 4:dy * 4 + 3] = w[:, ci, ky, kx]
    wlo0 = wlo[1].copy()
    for kx in range(7):
        for ci in range(16):
            for df in range(1, 4):
                for dy in range(32):
                    ky = 3 - df - dy
                    if 0 <= ky < 7:
                        wlo0[kx, ci * 8 + df,
                             dy * 4:dy * 4 + 3] += w[:, ci, ky, kx]
    wlo15 = wlo[4].copy()
    for kx in range(7):
        for ci in range(16):
            for dy in range(32):
                for ky in range(7):
                    r = 477 + dy + ky
                    if 512 <= r <= 514:
                        wlo15[kx, ci * 8 + (518 - r),
                              dy * 4:dy * 4 + 3] += w[:, ci, ky, kx]
    out['w_lo'] = _bf(wlo)
    out['w_lo0'] = _bf(wlo0)
    out['w_lo15'] = _bf(wlo15)
    b = np.asarray(inp['b_out'], np.float32)
    bl = np.zeros((128, 1), np.float32)
    for dy in range(32):
        bl[dy * 4:dy * 4 + 3, 0] = b
        bl[dy * 4 + 3, 0] = 20.0
    out['b_lo'] = bl

    for name, gs in [('ones8', 8), ('ones4', 4), ('ones2', 2)]:
        m = np.zeros((128, 128), np.float32)
        for i in range(128):
            blk = i // gs
            m[gs * blk:gs * blk + gs, i] = 1.0 / gs
        out[name] = m

    out['ident128'] = _bf(np.eye(128, dtype=np.float32))
    out['iota_oh'] = _bf(np.broadcast_to(np.arange(32, dtype=np.float32)[None, :],
                                         (128, 32)).copy())
    out['iota32'] = (np.arange(128)[:, None] % 32).astype(np.float32)
    return out


def input_specs():
    return {
        'x_pad': ((3, 518, 518), BF16),
        'ids': ((P,), BF16),
        'w_l0': ((2, 2, 96, 128), BF16),
        'w_d0': ((2, 3, 128, 128), BF16),
        'w_d1': ((2, 3, 128, 128), BF16),
        'w_d2': ((2, 3, 128, 128), BF16),
        'w_d3': ((2, 9, 128, 128), BF16),
        'w_u0': ((2, 2, 2, 2, 2, 128, 128), BF16),
        'w_u1': ((2, 2, 2, 128, 128), BF16),
        'w_u2': ((6, 128, 128), BF16),
        'w_u3': ((6, 128, 128), BF16),
        'w_lo': ((6, 7, 128, 128), BF16),
        'w_lo0': ((7, 128, 128), BF16),
        'w_lo15': ((7, 128, 128), BF16),
        'b_lo': ((128, 1), F32),
        'ones8': ((128, 128), F32),
        'ones4': ((128, 128), F32),
        'ones2': ((128, 128), F32),
        'ident128': ((128, 128), BF16),
        'iota_oh': ((128, 32), BF16),
        'iota32': ((128, 1), F32),
    }


# SBUF weight layouts: (sbuf_shape, einops from DRAM shape)
WSPEC = {
    'w_l0': ((96, 2, 2, 128), "a b k m -> k a b m"),
    'w_d0': ((128, 2, 3, 128), "a t k m -> k a t m"),
    'w_d1': ((128, 2, 3, 128), "a t k m -> k a t m"),
    'w_d2': ((128, 2, 3, 128), "a t k m -> k a t m"),
    'w_d3': ((128, 2, 9, 128), "h t k m -> k h t m"),
    'w_u0': ((128, 2, 2, 2, 2, 2, 128), "h a b va vb k m -> k h a b va vb m"),
    'w_u1': ((128, 2, 2, 2, 128), "b t o k m -> k b t o m"),
    'w_u2': ((128, 6, 128), "s k m -> k s m"),
    'w_u3': ((128, 6, 128), "s k m -> k s m"),
    'w_lo': ((128, 6, 7, 128), "e t k m -> k e t m"),
    'w_lo0': ((128, 7, 128), "t k m -> k t m"),
    'w_lo15': ((128, 7, 128), "t k m -> k t m"),
    'b_lo': ((128, 1), None),
    'ones8': ((128, 128), None),
    'ones4': ((128, 128), None),
    'ones2': ((128, 128), None),
    'ident128': ((128, 128), None),
    'iota_oh': ((128, 32), None),
    'iota32': ((128, 1), None),
}


def _inorm_relu(nc, sm, pp, interior, chunks, ones_lhs):
    """In-place instance-norm + relu. chunks: 2D APs [128, <=512]."""
    nchunk = len(chunks)
    stats = sm.tile([128, nchunk, 6], F32, tag="in_stats")
    for i, ch in enumerate(chunks):
        nc.vector.bn_stats(out=stats[:, i, :], in_=ch)
    mv = sm.tile([128, 2], F32, tag="in_mv")
    nc.vector.bn_aggr(out=mv, in_=stats)
    if ones_lhs is not None:
        m3 = sm.tile([128, 3], F32, tag="in_m3")
        nc.vector.tensor_copy(out=m3[:, 0:2], in_=mv)
        nc.vector.tensor_mul(m3[:, 2:3], mv[:, 0:1], mv[:, 0:1])
        cps = pp.tile([128, 3], F32, tag="in_comb")
        nc.tensor.matmul(cps, lhsT=ones_lhs, rhs=m3, start=True, stop=True)
        mbar = sm.tile([128, 3], F32, tag="in_mbar")
        nc.scalar.copy(out=mbar, in_=cps)
        m_col = mbar[:, 0:1]
        var = sm.tile([128, 1], F32, tag="in_var")
        nc.vector.tensor_add(var, mbar[:, 1:2], mbar[:, 2:3])
        mm = sm.tile([128, 1], F32, tag="in_mm")
        nc.vector.tensor_mul(mm, m_col, m_col)
        nc.vector.tensor_sub(var, var, mm)
    else:
        m_col = mv[:, 0:1]
        var = sm.tile([128, 1], F32, tag="in_var")
        nc.vector.tensor_copy(out=var, in_=mv[:, 1:2])
    nc.vector.tensor_scalar_add(var, var, EPS)
    sd = sm.tile([128, 1], F32, tag="in_sd")
    nc.scalar.sqrt(sd, var)
    s_col = sm.tile([128, 1], F32, tag="in_s")
    nc.vector.reciprocal(s_col, sd)
    b_col = sm.tile([128, 1], F32, tag="in_b")
    nc.vector.tensor_scalar(b_col, m_col, s_col, -1.0, AL.mult, AL.mult)
    ga = interior.shape[1]
    step = max(1, ga // 4)
    for g0 in range(0, ga, step):
        sl = interior[:, g0:min(g0 + step, ga), :]
        nc.scalar.activation(sl, sl, AF.Relu, bias=b_col, scale=s_col)


def build(nc, tc, ctx, upto='seg', dbg=None):
    spec = input_specs()
    din = {k: nc.dram_tensor(k, s, d, kind="ExternalInput")
           for k, (s, d) in spec.items()}
    dbg = dbg or {}
    stages = ['h0', 'd0', 'd1', 'd2', 'd3', 'u0', 'u1', 'u2', 'u3', 'f', 'seg']
    sidx = stages.index(upto)
    out_d = nc.dram_tensor("out", (3, P), F32, kind="ExternalOutput")

    sm = ctx.enter_context(tc.tile_pool(name="small", bufs=2))
    acts = ctx.enter_context(tc.tile_pool(name="acts", bufs=1))
    pp_s = ctx.enter_context(tc.tile_pool(name="psum_s", bufs=1, space="PSUM"))
    wpool = ctx.enter_context(tc.tile_pool(name="weights", bufs=1))

    def wload(name, pool=None):
        shape, rs = WSPEC[name]
        t = (pool or wpool).tile(list(shape), spec[name][1], tag="w_" + name)
        src = din[name][:]
        if rs is not None:
            src = src.rearrange(rs)
        nc.sync.dma_start(out=t, in_=src)
        return t

    ones8, ones4, ones2 = wload('ones8'), wload('ones4'), wload('ones2')

    def stage_done(name, tile_ap):
        if name in dbg:
            nc.sync.dma_start(out=dbg[name][:], in_=tile_ap)
        return sidx <= stages.index(name)

    # ================= L0 =================
    h0 = acts.tile([128, 65, 513], BF16, tag="slotA")
    nc.vector.memset(h0[:, 0:1, :], 0.0)
    nc.vector.memset(h0[:, :, 0:1], 0.0)
    with tc.tile_pool(name="lp_l0", bufs=1) as hrp, \
         tc.tile_pool(name="pp_l0", bufs=4, space="PSUM") as psp:
        w_l0 = wload('w_l0', hrp)
        xs = hrp.tile([96, 65, 518], BF16, tag="xs")
        nc.vector.memset(xs[:, 64:65, :], 0.0)
        nc.vector.memset(xs[:, :, 514:518], 0.0)
        xsr = xs.rearrange("(c rp s) g x -> s c rp g x", rp=8, s=4)
        for s in range(4):
            for c in range(3):
                nc.sync.dma_start(
                    out=xsr[s][c, :, 0:64, 0:518 - s],
                    in_=din['x_pad'][c, 0:512, s:518].rearrange(
                        "(g rp) x -> rp g x", rp=8))
            nc.sync.dma_start(
                out=xsr[s][:, 0:6, 64, 0:518 - s],
                in_=din['x_pad'][:, 512:518, s:518])
        for gi in range(64):
            ps = psp.tile([128, 512], F32, tag="ps")
            k = 0
            for dg in range(2):
                for kxp in range(2):
                    nc.tensor.matmul(ps, lhsT=w_l0[:, dg, kxp, :],
                                     rhs=xs[:, gi + dg, 4 * kxp:4 * kxp + 512],
                                     start=(k == 0), stop=(k == 3))
                    k += 1
            nc.scalar.copy(out=h0[:, 1 + gi, 1:513], in_=ps)
    _inorm_relu(nc, sm, pp_s, h0[:, 1:65, 1:513],
                [h0[:, 1 + i, 1:513] for i in range(64)], ones8)
    if stage_done('h0', h0):
        return din

    # ================= D0 (direct-AP) =================
    d0 = acts.tile([128, 65, 257], BF16, tag="slotB")
    nc.vector.memset(d0[:, 0:1, :], 0.0)
    nc.vector.memset(d0[:, :, 0:1], 0.0)
    with tc.tile_pool(name="lp_d0", bufs=1) as hrp, \
         tc.tile_pool(name="pp_d0", bufs=4, space="PSUM") as psp:
        w_d0 = wload('w_d0', hrp)
        for G in range(0, 64, 2):
            ps = psp.tile([128, 2, 256], F32, tag="ps")
            for kx in range(3):
                nc.tensor.matmul(ps, lhsT=w_d0[:, 0, kx, :],
                                 rhs=h0[:, 1 + G:3 + G, kx:kx + 511:2],
                                 start=(kx == 0), stop=False)
            for kx in range(3):
                nc.tensor.matmul(ps, lhsT=w_d0[:, 1, kx, :],
                                 rhs=h0[:, G:G + 2, kx:kx + 511:2],
                                 start=False, stop=(kx == 2))
            nc.scalar.copy(out=d0[:, 1 + G:3 + G, 1:257], in_=ps)
    _inorm_relu(nc, sm, pp_s, d0[:, 1:65, 1:257],
                [d0[:, 1 + i, 1:257] for i in range(64)], ones4)
    if stage_done('d0', d0):
        return din

    # ================= D1 (direct-AP) =================
    d1 = acts.tile([128, 65, 129], BF16, tag="slotA")
    nc.vector.memset(d1[:, 0:1, :], 0.0)
    nc.vector.memset(d1[:, :, 0:1], 0.0)
    with tc.tile_pool(name="lp_d1", bufs=1) as hrp, \
         tc.tile_pool(name="pp_d1", bufs=4, space="PSUM") as psp:
        w_d1 = wload('w_d1', hrp)
        for G in range(0, 64, 4):
            ps = psp.tile([128, 4, 128], F32, tag="ps")
            for kx in range(3):
                nc.tensor.matmul(ps, lhsT=w_d1[:, 0, kx, :],
                                 rhs=d0[:, 1 + G:5 + G, kx:kx + 255:2],
                                 start=(kx == 0), stop=False)
            for kx in range(3):
                nc.tensor.matmul(ps, lhsT=w_d1[:, 1, kx, :],
                                 rhs=d0[:, G:G + 4, kx:kx + 255:2],
                                 start=False, stop=(kx == 2))
            nc.scalar.copy(out=d1[:, 1 + G:5 + G, 1:129], in_=ps)
    _inorm_relu(nc, sm, pp_s, d1[:, 1:65, 1:129],
                [d1[:, 1 + i, 1:129] for i in range(64)], ones2)
    if stage_done('d1', d1):
        return din

    # ================= D2 (direct-AP) =================
    d2 = acts.tile([128, 65, 65], BF16, tag="slotB")
    nc.vector.memset(d2[:, 0:1, :], 0.0)
    nc.vector.memset(d2[:, :, 0:1], 0.0)
    with tc.tile_pool(name="lp_d2", bufs=1) as hrp, \
         tc.tile_pool(name="pp_d2", bufs=4, space="PSUM") as psp:
        w_d2 = wload('w_d2', hrp)
        for Y0 in range(0, 64, 8):
            ps = psp.tile([128, 8, 64], F32, tag="ps")
            for kx in range(3):
                nc.tensor.matmul(ps, lhsT=w_d2[:, 0, kx, :],
                                 rhs=d1[:, 1 + Y0:9 + Y0, kx:kx + 127:2],
                                 start=(kx == 0), stop=False)
            for kx in range(3):
                nc.tensor.matmul(ps, lhsT=w_d2[:, 1, kx, :],
                                 rhs=d1[:, Y0:Y0 + 8, kx:kx + 127:2],
                                 start=False, stop=(kx == 2))
            nc.scalar.copy(out=d2[:, 1 + Y0:9 + Y0, 1:65], in_=ps)
    _inorm_relu(nc, sm, pp_s, d2[:, 1:65, 1:65],
                [d2[:, 1 + i, 1:65] for i in range(64)], None)
    if stage_done('d2', d2):
        return din

    # ================= D3 =================
    d3 = acts.tile([128, 2, 33, 33], BF16, tag="slotA")
    nc.vector.memset(d3[:, :, 32:33, :], 0.0)
    nc.vector.memset(d3[:, :, :, 32:33], 0.0)
    with tc.tile_pool(name="lp_d3", bufs=1) as hrp, \
         tc.tile_pool(name="pp_d3", bufs=4, space="PSUM") as psp:
        w_d3 = wload('w_d3', hrp)
        for h in range(2):
            for blk in range(2):
                ps = psp.tile([128, 16, 32], F32, tag="ps")
                first = True
                for ky in range(3):
                    for kx in range(3):
                        s0 = 32 * blk + ky
                        rhs = d2[:, s0:s0 + 31:2, kx:kx + 63:2]
                        nc.tensor.matmul(ps, lhsT=w_d3[:, h, ky * 3 + kx, :],
                                         rhs=rhs, start=first,
                                         stop=(ky == 2 and kx == 2))
                        first = False
                nc.scalar.copy(out=d3[:, h, 16 * blk:16 * blk + 16, 0:32], in_=ps)
    for h in range(2):
        _inorm_relu(nc, sm, pp_s, d3[:, h, 0:32, 0:32],
                    [d3[:, h, i, 0:32] for i in range(32)], None)
    if stage_done('d3', d3):
        return din

    # ================= U0 =================
    u0 = acts.tile([128, 65, 65], BF16, tag="slotB")
    nc.vector.memset(u0[:, 64:65, :], 0.0)
    nc.vector.memset(u0[:, :, 64:65], 0.0)
    with tc.tile_pool(name="lp_u0", bufs=1) as hrp, \
         tc.tile_pool(name="pp_u0", bufs=4, space="PSUM") as psp:
        w_u0 = wload('w_u0', hrp)
        for a in range(2):
            for b in range(2):
                for blk in range(2):
                    ps = psp.tile([128, 16, 32], F32, tag="ps")
                    mms = [(h, va, vb) for h in range(2) for va in range(2)
                           for vb in range(2)
                           if 0 <= a + 1 - 2 * va < 3 and 0 <= b + 1 - 2 * vb < 3]
                    for mi, (h, va, vb) in enumerate(mms):
                        rhs = d3[:, h, 16 * blk + va:16 * blk + va + 16,
                                 vb:vb + 32]
                        nc.tensor.matmul(ps, lhsT=w_u0[:, h, a, b, va, vb, :],
                                         rhs=rhs, start=(mi == 0),
                                         stop=(mi == len(mms) - 1))
                    nc.scalar.copy(
                        out=u0[:, 32 * blk + a:32 * blk + a + 31:2, b:b + 63:2],
                        in_=ps)
    _inorm_relu(nc, sm, pp_s, u0[:, 0:64, 0:64],
                [u0[:, i, 0:64] for i in range(64)], None)
    if stage_done('u0', u0):
        return din

    # ================= U1 =================
    u1 = acts.tile([128, 65, 65, 2], BF16, tag="slotA")
    nc.vector.memset(u1[:, 64:65, :, :], 0.0)
    nc.vector.memset(u1[:, :, 64:65, :], 0.0)
    with tc.tile_pool(name="lp_u1", bufs=1) as hrp, \
         tc.tile_pool(name="pp_u1", bufs=4, space="PSUM") as psp:
        w_u1 = wload('w_u1', hrp)
        for b in range(2):
            for blk in range(8):
                ps = psp.tile([128, 8, 64], F32, tag="ps")
                mms = [(t, hoff) for t in range(2) for hoff in range(2)]
                for mi, (t, hoff) in enumerate(mms):
                    rhs = u0[:, 8 * blk + t:8 * blk + t + 8, hoff:hoff + 64]
                    nc.tensor.matmul(ps, lhsT=w_u1[:, b, t, hoff, :], rhs=rhs,
                                     start=(mi == 0), stop=(mi == len(mms) - 1))
                nc.scalar.copy(out=u1[:, 8 * blk:8 * blk + 8, 0:64, b], in_=ps)
    u1x = u1.rearrange("p g x b -> p g (x b)")
    _inorm_relu(nc, sm, pp_s, u1x[:, 0:64, 0:128],
                [u1x[:, i, 0:128] for i in range(64)], ones2)
    if stage_done('u1', u1):
        return din

    # ================= U2 (direct-AP) =================
    u2 = acts.tile([128, 65, 258], BF16, tag="slotB")
    nc.vector.memset(u2[:, 64:65, :], 0.0)
    nc.vector.memset(u2[:, :, 0:1], 0.0)
    nc.vector.memset(u2[:, :, 257:258], 0.0)
    u1x = u1.rearrange("p g x b -> p g (x b)")
    with tc.tile_pool(name="lp_u2", bufs=1) as hrp, \
         tc.tile_pool(name="pp_u2", bufs=4, space="PSUM") as psp:
        w_u2 = wload('w_u2', hrp)
        for G in range(0, 64, 4):
            pse = psp.tile([128, 4, 128], F32, tag="ps")
            nc.tensor.matmul(pse, lhsT=w_u2[:, 0, :],
                             rhs=u1x[:, G:G + 4, 0:128], start=True, stop=False)
            nc.tensor.matmul(pse, lhsT=w_u2[:, 1, :],
                             rhs=u1x[:, G + 1:G + 5, 0:128], start=False,
                             stop=True)
            nc.scalar.copy(out=u2[:, G:G + 4, 1:257:2], in_=pse)
            pso = psp.tile([128, 4, 128], F32, tag="ps")
            nc.tensor.matmul(pso, lhsT=w_u2[:, 2, :],
                             rhs=u1x[:, G:G + 4, 1:129], start=True, stop=False)
            nc.tensor.matmul(pso, lhsT=w_u2[:, 3, :],
                             rhs=u1x[:, G:G + 4, 0:128], start=False,
                             stop=False)
            nc.tensor.matmul(pso, lhsT=w_u2[:, 4, :],
                             rhs=u1x[:, G + 1:G + 5, 1:129], start=False,
                             stop=False)
            nc.tensor.matmul(pso, lhsT=w_u2[:, 5, :],
                             rhs=u1x[:, G + 1:G + 5, 0:128], start=False,
                             stop=True)
            nc.scalar.copy(out=u2[:, G:G + 4, 2:258:2], in_=pso)
    _inorm_relu(nc, sm, pp_s, u2[:, 0:64, 1:257],
                [u2[:, i, 1:257] for i in range(64)], ones4)
    if stage_done('u2', u2):
        return din

    # ================= U3 (direct-AP) =================
    u3 = acts.tile([128, 64, 520], BF16, tag="slotA")
    with tc.tile_pool(name="lp_u3", bufs=1) as hrp, \
         tc.tile_pool(name="pp_u3", bufs=4, space="PSUM") as psp:
        w_u3 = wload('w_u3', hrp)
        for G in range(0, 64, 2):
            pse = psp.tile([128, 2, 256], F32, tag="ps")
            nc.tensor.matmul(pse, lhsT=w_u3[:, 0, :],
                             rhs=u2[:, G:G + 2, 1:257], start=True, stop=False)
            nc.tensor.matmul(pse, lhsT=w_u3[:, 1, :],
                             rhs=u2[:, G + 1:G + 3, 1:257], start=False,
                             stop=True)
            nc.scalar.copy(out=u3[:, G:G + 2, 4:516:2], in_=pse)
            pso = psp.tile([128, 2, 256], F32, tag="ps")
            nc.tensor.matmul(pso, lhsT=w_u3[:, 2, :],
                             rhs=u2[:, G:G + 2, 2:258], start=True, stop=False)
            nc.tensor.matmul(pso, lhsT=w_u3[:, 3, :],
                             rhs=u2[:, G:G + 2, 1:257], start=False, stop=False)
            nc.tensor.matmul(pso, lhsT=w_u3[:, 4, :],
                             rhs=u2[:, G + 1:G + 3, 2:258], start=False,
                             stop=False)
            nc.tensor.matmul(pso, lhsT=w_u3[:, 5, :],
                             rhs=u2[:, G + 1:G + 3, 1:257], start=False,
                             stop=True)
            nc.scalar.copy(out=u3[:, G:G + 2, 5:517:2], in_=pso)
    _inorm_relu(nc, sm, pp_s, u3[:, :, 4:516],
                [u3[:, i, 4:516] for i in range(64)], ones8)
    for dst, src in [(3, 5), (2, 6), (1, 7), (516, 514), (517, 513), (518, 512)]:
        nc.scalar.copy(out=u3[:, :, dst:dst + 1], in_=u3[:, :, src:src + 1])
    if stage_done('u3', u3):
        return din

    # ================= L_out (direct-AP; M=(dy*4+q), q=3 rows -> ones) =====
    f_nat = acts.tile([128, 16, 512], BF16, tag="slotB")
    with tc.tile_pool(name="lp_lo", bufs=1) as hrp, \
         tc.tile_pool(name="pp_lo", bufs=6, space="PSUM") as psp:
        w_lo = wload('w_lo', hrp)
        w_lo0 = wload('w_lo0', hrp)
        w_lo15 = wload('w_lo15', hrp)
        b_lo = wload('b_lo', hrp)
        for grp in range(16):
            ps = psp.tile([128, 512], F32, tag="ps")
            taps = []
            for e in range(6):
                gp = 4 * grp + e - 1
                if gp < 0 or gp > 63:
                    continue
                for kx in range(7):
                    if grp == 0 and e == 1:
                        lh = w_lo0[:, kx, :]
                    elif grp == 15 and e == 4:
                        lh = w_lo15[:, kx, :]
                    else:
                        lh = w_lo[:, e, kx, :]
                    taps.append((lh, gp, kx))
            for i, (lh, gp, kx) in enumerate(taps):
                nc.tensor.matmul(ps, lhsT=lh, rhs=u3[:, gp, kx + 1:kx + 513],
                                 start=(i == 0), stop=(i == len(taps) - 1))
            nc.scalar.activation(f_nat[:, grp, :], ps, AF.Tanh, bias=b_lo,
                                 scale=1.0)
    if stage_done('f', f_nat):
        return din

    # ================= segment mean =================
    segp = ctx.enter_context(tc.tile_pool(name="segbig", bufs=1))
    ident128 = wload('ident128', segp)
    iota_oh = wload('iota_oh', segp)
    iota32 = wload('iota32', segp)

    # ids natural layout (independent of f; scheduled early)
    ids_nat = segp.tile([128, 4, 512], BF16)  # [ylow, rb, x]
    nc.sync.dma_start(out=ids_nat,
                      in_=din['ids'].rearrange("(rb y x) -> y rb x", y=128,
                                               x=512))

    sums_sb = sm.tile([4, 32], F32, tag="sums_sb")
    nc.vector.memset(sums_sb, 0.0)
    idsT_t = segp.tile([128, 4, 4, 128], BF16)
    with tc.tile_pool(name="pp_tr", bufs=2, space="PSUM") as ppt, \
         tc.tile_pool(name="pp_sums", bufs=2, space="PSUM") as pps, \
         tc.tile_pool(name="segoh", bufs=6) as ohp, \
         tc.tile_pool(name="segft", bufs=6) as ftp:
        # idsT_t[p, xb, rb, ylow] = ids[Y=128*rb+ylow, 128*xb+p]
        for rb in range(4):
            for xb in range(4):
                pt = ppt.tile([128, 128], BF16)
                nc.tensor.transpose(pt,
                                    ids_nat[:, rb, 128 * xb:128 * xb + 128],
                                    ident128)
                nc.scalar.copy(out=idsT_t[:, xb, rb, :], in_=pt)

        # stage A: chunk (grp, xb, dy) = pixels (Y=32*grp+dy, x in xb-block):
        # psum[4,32] += fT[:, 4*dy:4*dy+4]^T @ oh[dy]
        idsv = idsT_t.rearrange("p xb rb y -> p xb (rb y)")  # [128, 4, 512]
        for j in range(64):
            grp, xb = j // 4, j % 4
            ptr = ppt.tile([128, 128], BF16)
            nc.tensor.transpose(ptr, f_nat[:, grp, 128 * xb:128 * (xb + 1)],
                                ident128)
            fT = ftp.tile([128, 128], BF16)
            nc.scalar.copy(out=fT, in_=ptr)
            oh = ohp.tile([128, 32, 32], BF16)
            ids_sl = idsv[:, xb, 32 * grp:32 * grp + 32]  # [128, 32] over dy
            nc.vector.tensor_tensor(
                out=oh,
                in0=ids_sl.unsqueeze(2).broadcast_to([128, 32, 32]),
                in1=iota_oh.unsqueeze(1).broadcast_to([128, 32, 32]),
                op=AL.is_equal)
            ps = pps.tile([4, 32], F32)
            for dy in range(32):
                nc.tensor.matmul(ps, lhsT=fT[:, 4 * dy:4 * dy + 4],
                                 rhs=oh[:, dy, :],
                                 start=(dy == 0), stop=(dy == 31))
            nc.vector.tensor_add(sums_sb, sums_sb, ps)

    if 'sums' in dbg:
        nc.sync.dma_start(out=dbg['sums'][:], in_=sums_sb)
    sums32 = sm.tile([32, 32], F32, tag="sums32")
    nc.vector.memset(sums32, 0.0)
    nc.vector.tensor_copy(out=sums32[0:4, :], in_=sums_sb)
    sumsT = sm.tile([32, 32], F32, tag="sumsT")
    nc.vector.transpose(sumsT, sums32)
    cntm = sm.tile([32, 1], F32, tag="cntm")
    nc.vector.tensor_scalar_max(cntm, sumsT[:, 3:4], 1.0)
    rcnt = sm.tile([32, 1], F32, tag="rcnt")
    nc.vector.reciprocal(rcnt, cntm)
    means_bf = sm.tile([32, 3], BF16, tag="means_bf")
    nc.vector.tensor_scalar_mul(means_bf, sumsT[:, 0:3], rcnt)
    if 'means' in dbg:
        nc.sync.dma_start(out=dbg['means'][:], in_=means_bf)
    bd = sm.tile([128, 12], BF16, tag="bd")
    nc.vector.memset(bd, 0.0)
    for s in range(4):
        nc.sync.dma_start(out=bd[32 * s:32 * s + 32, 3 * s:3 * s + 3],
                          in_=means_bf)

    # stage B: out[c, p] = means[c, id[p]] via block-diag one-hot matmul
    ids_q = din['ids'].rearrange("(q n) -> q n", q=4)
    out_r = out_d.rearrange("c (q x) -> q c x", q=4)
    with tc.tile_pool(name="seggat", bufs=6) as gat, \
         tc.tile_pool(name="segstg", bufs=3) as stgp, \
         tc.tile_pool(name="psumg", bufs=6, space="PSUM") as ppg:
        for t in range(32):
            ids_rep = gat.tile([128, 2048], BF16)
            for q in range(4):
                nc.sync.dma_start(
                    out=ids_rep[32 * q:32 * q + 32, :],
                    in_=ids_q[q:q + 1, t * 2048:(t + 1) * 2048].broadcast_to(
                        [32, 2048]))
            oh_g = gat.tile([128, 2048], BF16)
            nc.vector.tensor_scalar(out=oh_g, in0=ids_rep, scalar1=iota32,
                                    scalar2=None, op0=AL.is_equal)
            stg = stgp.tile([12, 2048], F32)
            for w in range(4):
                psg = ppg.tile([12, 512], F32)
                nc.tensor.matmul(psg, lhsT=bd,
                                 rhs=oh_g[:, w * 512:(w + 1) * 512],
                                 start=True, stop=True)
                if w % 2 == 0:
                    nc.scalar.copy(out=stg[:, 512 * w:512 * (w + 1)], in_=psg)
                else:
                    nc.vector.tensor_copy(out=stg[:, 512 * w:512 * (w + 1)],
                                          in_=psg)
            for q in range(4):
                eng = nc.sync if q < 2 else nc.scalar
                eng.dma_start(
                    out=out_r[q, :, 2048 * t:2048 * (t + 1)],
                    in_=stg[3 * q:3 * q + 3, :])
    return din


# ======================================================================
# public entry: kernel(**inputs) with FULL batch inputs, 8-core SPMD
# ======================================================================
import concourse.bacc as _bacc
from concourse import bass_utils as _bass_utils

_CACHE = {}


def _get_nc():
    if 'nc' not in _CACHE:
        nc = _bacc.Bacc("TRN2", target_bir_lowering=False)
        with contextlib.ExitStack() as ctx:
            tc = ctx.enter_context(tile.TileContext(nc, pool_alloc_mode="queue"))
            build(nc, tc, ctx, upto='seg')
        nc.compile()
        _CACHE['nc'] = nc
    return _CACHE['nc']


def kernel(**inputs):
    nc = _get_nc()
    x = np.asarray(inputs['x'])
    ids = np.asarray(inputs['instance_map'])
    B = x.shape[0]
    shared = None
    in_maps = []
    for bi in range(B):
        inp0 = {k: v for k, v in inputs.items()}
        inp0['x'] = x[bi]
        inp0['instance_map'] = ids[bi]
        if shared is None:
            m = pack_inputs(inp0)
            shared = {k: v for k, v in m.items() if k not in ('x_pad', 'ids')}
        else:
            m = dict(shared)
            xp = np.pad(np.asarray(inp0['x'], np.float32), ((0, 0), (3, 3), (3, 3)),
                        mode='reflect')
            m['x_pad'] = _bf(xp)
            m['ids'] = _bf(np.asarray(inp0['instance_map']).reshape(-1).astype(
                np.float32))
        in_maps.append(m)
    res = _bass_utils.run_bass_kernel_spmd(nc, in_maps, core_ids=list(range(B)))
    out = np.stack([res.results[i]['out'].reshape(3, 512, 512) for i in range(B)])
    return out.astype(np.float32)


def kernel_traced(**inputs):
    """Like kernel() but with NTFF tracing; returns (out, exec_time_ns, profile)."""
    nc = _get_nc()
    x = np.asarray(inputs['x'])
    ids = np.asarray(inputs['instance_map'])
    B = x.shape[0]
    shared = None
    in_maps = []
    for bi in range(B):
        inp0 = {k: v for k, v in inputs.items()}
        inp0['x'] = x[bi]
        inp0['instance_map'] = ids[bi]
        if shared is None:
            m = pack_inputs(inp0)
            shared = {k: v for k, v in m.items() if k not in ('x_pad', 'ids')}
        else:
            m = dict(shared)
            xp = np.pad(np.asarray(inp0['x'], np.float32), ((0, 0), (3, 3), (3, 3)),
                        mode='reflect')
            m['x_pad'] = _bf(xp)
            m['ids'] = _bf(np.asarray(inp0['instance_map']).reshape(-1).astype(
                np.float32))
        in_maps.append(m)
    res = _bass_utils.run_bass_kernel_spmd(nc, in_maps, core_ids=list(range(B)),
                                           trace=True)
    out = np.stack([res.results[i]['out'].reshape(3, 512, 512) for i in range(B)])
    return out.astype(np.float32), res.exec_time_ns, res



# revision 41
# speedup vs baseline: 1.0270x; 1.0270x over previous
"""Encoder kernel builder for nn_Encoder (conv stack + segment-mean) on TRN2.
See layout notes in docstring history; key contracts:
  h0_nat [128=(co16*8+dy), 64, 513]; d0_nat [128=(co32*4+dy), 64, 257]
  d1_nat [128=(co64*2+dy), 64, 129]; d2_nat [128, 65, 65] (tl pad)
  d3_nat [128, 2, 33, 33] (br pad);  u0_nat [128, 65, 65] (br pad)
  u1_nat [128=(co64*2+df), 65, 65, 2]; u2_nat [128=(co32*4+df), 65, 129, 2]
  u3_nat [128=(co16*8+df), 64, 260, 2] (X-linear idx = X+4, reflect cols)
  f_nat  [96=(co3*32+dy), 16, 512]
"""
import contextlib
import numpy as np
import ml_dtypes
import concourse.bass as bass
import concourse.tile as tile
from concourse import mybir

BF16 = mybir.dt.bfloat16
F32 = mybir.dt.float32
EPS = 1e-5
AL = mybir.AluOpType
AF = mybir.ActivationFunctionType
P = 512 * 512


def _bf(x):
    return np.ascontiguousarray(x.astype(ml_dtypes.bfloat16))


def _tc_entry(w, a, b, va, vb):
    ka = a + 1 - 2 * va
    kb = b + 1 - 2 * vb
    if 0 <= ka < 3 and 0 <= kb < 3:
        return w[:, :, ka, kb]
    return None


def pack_inputs(inp):
    out = {}
    x = np.asarray(inp['x'], np.float32)
    out['x_pad'] = _bf(np.pad(x, ((0, 0), (3, 3), (3, 3)), mode='reflect'))
    out['ids'] = _bf(np.asarray(inp['instance_map']).reshape(-1).astype(np.float32))

    # L0 direct-AP: K=96=(c,rp,s 4 x-shift slots), taps (dg,kxp); kx=4*kxp+s
    w = np.asarray(inp['w_in'], np.float32)  # [16, 3, 7, 7]
    wl = np.zeros((2, 2, 96, 128), np.float32)
    for dg in range(2):
        for kxp in range(2):
            for c in range(3):
                for rp in range(8):
                    for s in range(4):
                        kx = 4 * kxp + s
                        if kx >= 7:
                            continue
                        for dy in range(8):
                            ky = 8 * dg + rp - dy
                            if 0 <= ky < 7:
                                wl[dg, kxp, c * 32 + rp * 4 + s, dy::8] = \
                                    w[:, c, ky, kx]
    out['w_l0'] = _bf(wl)

    # D-stages: direct-AP taps [2(main/corr), 3 kx, 128 K=(ci,df), 128 M=(co,dy)]
    w = np.asarray(inp['w_d0'], np.float32)  # [32, 16, 3, 3]
    wd = np.zeros((2, 3, 128, 128), np.float32)
    for kx in range(3):
        for ci in range(16):
            for df in range(8):
                for dy in range(4):
                    ky = df - 2 * dy + 1
                    if 0 <= ky < 3:
                        wd[0, kx, ci * 8 + df, dy::4] = w[:, ci, ky, kx]
            wd[1, kx, ci * 8 + 7, 0::4] = w[:, ci, 0, kx]
    out['w_d0'] = _bf(wd)

    w = np.asarray(inp['w_d1'], np.float32)  # [64, 32, 3, 3]
    wd = np.zeros((2, 3, 128, 128), np.float32)
    for kx in range(3):
        for ci in range(32):
            for df in range(4):
                for dy in range(2):
                    ky = df - 2 * dy + 1
                    if 0 <= ky < 3:
                        wd[0, kx, ci * 4 + df, dy::2] = w[:, ci, ky, kx]
            wd[1, kx, ci * 4 + 3, 0::2] = w[:, ci, 0, kx]
    out['w_d1'] = _bf(wd)

    w = np.asarray(inp['w_d2'], np.float32)  # [128, 64, 3, 3]
    wd = np.zeros((2, 3, 128, 128), np.float32)
    for kx in range(3):
        for ci in range(64):
            for df in range(2):
                wd[0, kx, ci * 2 + df, :] = w[:, ci, df + 1, kx]
            wd[1, kx, ci * 2 + 1, :] = w[:, ci, 0, kx]
    out['w_d2'] = _bf(wd)

    w = np.asarray(inp['w_d3'], np.float32)
    wd3 = np.zeros((2, 9, 128, 128), np.float32)
    for h in range(2):
        for ky in range(3):
            for kx in range(3):
                wd3[h, ky * 3 + kx] = w[128 * h:128 * h + 128, :, ky, kx].T
    out['w_d3'] = _bf(wd3)

    w = np.asarray(inp['w_u0'], np.float32)
    wu0 = np.zeros((2, 2, 2, 2, 2, 128, 128), np.float32)
    for h in range(2):
        for a in range(2):
            for b in range(2):
                for va in range(2):
                    for vb in range(2):
                        e = _tc_entry(w[128 * h:128 * h + 128], a, b, va, vb)
                        if e is not None:
                            wu0[h, a, b, va, vb] = e
    out['w_u0'] = _bf(wu0)

    w = np.asarray(inp['w_u1'], np.float32)
    wu1 = np.zeros((2, 2, 2, 128, 128), np.float32)
    for b in range(2):
        for t in range(2):
            for hoff in range(2):
                for df in range(2):
                    a = df % 2
                    va = t - (df - a) // 2
                    if va not in (0, 1):
                        continue
                    e = _tc_entry(w, a, b, va, hoff)
                    if e is not None:
                        wu1[b, t, hoff, :, df::2] = e
    out['w_u1'] = _bf(wu1)

    # U2/U3: direct-AP tconv classes [6, 128 K=(ci,df_in), 128 M=(co,dfY)]
    w = np.asarray(inp['w_u2'], np.float32)  # [64, 32, 3, 3] torch (ci, co)
    wu = np.zeros((6, 128, 128), np.float32)
    pm = [(0, 0, 1), (1, 1, 0), (0, 1, 2), (1, 2, 1), (1, 3, 2)]
    pc = [(0, 3, 0)]
    for cls, pairs, kx in [(0, pm, 1), (1, pc, 1), (2, pm, 0), (3, pm, 2),
                           (4, pc, 0), (5, pc, 2)]:
        for (df1, dfY, ky) in pairs:
            for ci in range(64):
                wu[cls, ci * 2 + df1, dfY::4] = w[ci, :, ky, kx]
    out['w_u2'] = _bf(wu)

    w = np.asarray(inp['w_u3'], np.float32)  # [32, 16, 3, 3]
    wu = np.zeros((6, 128, 128), np.float32)
    pm = [(0, 0, 1), (1, 2, 1), (2, 4, 1), (3, 6, 1), (1, 1, 0), (0, 1, 2),
          (2, 3, 0), (1, 3, 2), (3, 5, 0), (2, 5, 2), (3, 7, 2)]
    pc = [(0, 7, 0)]
    for cls, pairs, kx in [(0, pm, 1), (1, pc, 1), (2, pm, 0), (3, pm, 2),
                           (4, pc, 0), (5, pc, 2)]:
        for (df2, dfY, ky) in pairs:
            for ci in range(32):
                wu[cls, ci * 4 + df2, dfY::8] = w[ci, :, ky, kx]
    out['w_u3'] = _bf(wu)

    # Lout: direct-AP 7x7 taps; M = (dy*4 + q), q=3 rows produce ones via bias
    w = np.asarray(inp['w_out'], np.float32)  # [3, 16, 7, 7]
    wlo = np.zeros((6, 7, 128, 128), np.float32)
    for e in range(6):
        for kx in range(7):
            for ci in range(16):
                for df in range(8):
                    for dy in range(32):
                        ky = 8 * (e - 1) + df - dy + 3
                        if 0 <= ky < 7:
                            wlo[e, kx, ci * 8 + df,
                                dy * 4:dy * 4 + 3] = w[:, ci, ky, kx]
    wlo0 = wlo[1].copy()
    for kx in range(7):
        for ci in range(16):
            for df in range(1, 4):
                for dy in range(32):
                    ky = 3 - df - dy
                    if 0 <= ky < 7:
                        wlo0[kx, ci * 8 + df,
                             dy * 4:dy * 4 + 3] += w[:, ci, ky, kx]
    wlo15 = wlo[4].copy()
    for kx in range(7):
        for ci in range(16):
            for dy in range(32):
                for ky in range(7):
                    r = 477 + dy + ky
                    if 512 <= r <= 514:
                        wlo15[kx, ci * 8 + (518 - r),
                              dy * 4:dy * 4 + 3] += w[:, ci, ky, kx]
    out['w_lo'] = _bf(wlo)
    out['w_lo0'] = _bf(wlo0)
    out['w_lo15'] = _bf(wlo15)
    b = np.asarray(inp['b_out'], np.float32)
    bl = np.zeros((128, 1), np.float32)
    for dy in range(32):
        bl[dy * 4:dy * 4 + 3, 0] = b
        bl[dy * 4 + 3, 0] = 20.0
    out['b_lo'] = bl

    for name, gs in [('ones8', 8), ('ones4', 4), ('ones2', 2)]:
        m = np.zeros((128, 128), np.float32)
        for i in range(128):
            blk = i // gs
            m[gs * blk:gs * blk + gs, i] = 1.0 / gs
        out[name] = m

    out['ident128'] = _bf(np.eye(128, dtype=np.float32))
    out['iota_oh'] = _bf(np.broadcast_to(np.arange(32, dtype=np.float32)[None, :],
                                         (128, 32)).copy())
    out['iota32'] = (np.arange(128)[:, None] % 32).astype(np.float32)
    return out


def input_specs():
    return {
        'x_pad': ((3, 518, 518), BF16),
        'ids': ((P,), BF16),
        'w_l0': ((2, 2, 96, 128), BF16),
        'w_d0': ((2, 3, 128, 128), BF16),
        'w_d1': ((2, 3, 128, 128), BF16),
        'w_d2': ((2, 3, 128, 128), BF16),
        'w_d3': ((2, 9, 128, 128), BF16),
        'w_u0': ((2, 2, 2, 2, 2, 128, 128), BF16),
        'w_u1': ((2, 2, 2, 128, 128), BF16),
        'w_u2': ((6, 128, 128), BF16),
        'w_u3': ((6, 128, 128), BF16),
        'w_lo': ((6, 7, 128, 128), BF16),
        'w_lo0': ((7, 128, 128), BF16),
        'w_lo15': ((7, 128, 128), BF16),
        'b_lo': ((128, 1), F32),
        'ones8': ((128, 128), F32),
        'ones4': ((128, 128), F32),
        'ones2': ((128, 128), F32),
        'ident128': ((128, 128), BF16),
        'iota_oh': ((128, 32), BF16),
        'iota32': ((128, 1), F32),
    }


# SBUF weight layouts: (sbuf_shape, einops from DRAM shape)
WSPEC = {
    'w_l0': ((96, 2, 2, 128), "a b k m -> k a b m"),
    'w_d0': ((128, 2, 3, 128), "a t k m -> k a t m"),
    'w_d1': ((128, 2, 3, 128), "a t k m -> k a t m"),
    'w_d2': ((128, 2, 3, 128), "a t k m -> k a t m"),
    'w_d3': ((128, 2, 9, 128), "h t k m -> k h t m"),
    'w_u0': ((128, 2, 2, 2, 2, 2, 128), "h a b va vb k m -> k h a b va vb m"),
    'w_u1': ((128, 2, 2, 2, 128), "b t o k m -> k b t o m"),
    'w_u2': ((128, 6, 128), "s k m -> k s m"),
    'w_u3': ((128, 6, 128), "s k m -> k s m"),
    'w_lo': ((128, 6, 7, 128), "e t k m -> k e t m"),
    'w_lo0': ((128, 7, 128), "t k m -> k t m"),
    'w_lo15': ((128, 7, 128), "t k m -> k t m"),
    'b_lo': ((128, 1), None),
    'ones8': ((128, 128), None),
    'ones4': ((128, 128), None),
    'ones2': ((128, 128), None),
    'ident128': ((128, 128), None),
    'iota_oh': ((128, 32), None),
    'iota32': ((128, 1), None),
}


def _inorm_relu(nc, sm, pp, interior, chunks, ones_lhs):
    """In-place instance-norm + relu. chunks: 2D APs [128, <=512]."""
    nchunk = len(chunks)
    stats = sm.tile([128, nchunk, 6], F32, tag="in_stats")
    for i, ch in enumerate(chunks):
        nc.vector.bn_stats(out=stats[:, i, :], in_=ch)
    mv = sm.tile([128, 2], F32, tag="in_mv")
    nc.vector.bn_aggr(out=mv, in_=stats)
    if ones_lhs is not None:
        m3 = sm.tile([128, 3], F32, tag="in_m3")
        nc.vector.tensor_copy(out=m3[:, 0:2], in_=mv)
        nc.vector.tensor_mul(m3[:, 2:3], mv[:, 0:1], mv[:, 0:1])
        cps = pp.tile([128, 3], F32, tag="in_comb")
        nc.tensor.matmul(cps, lhsT=ones_lhs, rhs=m3, start=True, stop=True)
        mbar = sm.tile([128, 3], F32, tag="in_mbar")
        nc.scalar.copy(out=mbar, in_=cps)
        m_col = mbar[:, 0:1]
        var = sm.tile([128, 1], F32, tag="in_var")
        nc.vector.tensor_add(var, mbar[:, 1:2], mbar[:, 2:3])
        mm = sm.tile([128, 1], F32, tag="in_mm")
        nc.vector.tensor_mul(mm, m_col, m_col)
        nc.vector.tensor_sub(var, var, mm)
    else:
        m_col = mv[:, 0:1]
        var = sm.tile([128, 1], F32, tag="in_var")
        nc.vector.tensor_copy(out=var, in_=mv[:, 1:2])
    nc.vector.tensor_scalar_add(var, var, EPS)
    sd = sm.tile([128, 1], F32, tag="in_sd")
    nc.scalar.sqrt(sd, var)
    s_col = sm.tile([128, 1], F32, tag="in_s")
    nc.vector.reciprocal(s_col, sd)
    b_col = sm.tile([128, 1], F32, tag="in_b")
    nc.vector.tensor_scalar(b_col, m_col, s_col, -1.0, AL.mult, AL.mult)
    ga = interior.shape[1]
    step = max(1, ga // 4)
    for g0 in range(0, ga, step):
        sl = interior[:, g0:min(g0 + step, ga), :]
        nc.scalar.activation(sl, sl, AF.Relu, bias=b_col, scale=s_col)


def build(nc, tc, ctx, upto='seg', dbg=None):
    spec = input_specs()
    din = {k: nc.dram_tensor(k, s, d, kind="ExternalInput")
           for k, (s, d) in spec.items()}
    dbg = dbg or {}
    stages = ['h0', 'd0', 'd1', 'd2', 'd3', 'u0', 'u1', 'u2', 'u3', 'f', 'seg']
    sidx = stages.index(upto)
    out_d = nc.dram_tensor("out", (3, P), F32, kind="ExternalOutput")

    sm = ctx.enter_context(tc.tile_pool(name="small", bufs=2))
    acts = ctx.enter_context(tc.tile_pool(name="acts", bufs=1))
    pp_s = ctx.enter_context(tc.tile_pool(name="psum_s", bufs=1, space="PSUM"))
    wpool = ctx.enter_context(tc.tile_pool(name="weights", bufs=1))

    def wload(name, pool=None):
        shape, rs = WSPEC[name]
        t = (pool or wpool).tile(list(shape), spec[name][1], tag="w_" + name)
        src = din[name][:]
        if rs is not None:
            src = src.rearrange(rs)
        nc.sync.dma_start(out=t, in_=src)
        return t

    ones8, ones4, ones2 = wload('ones8'), wload('ones4'), wload('ones2')

    def stage_done(name, tile_ap):
        if name in dbg:
            nc.sync.dma_start(out=dbg[name][:], in_=tile_ap)
        return sidx <= stages.index(name)

    # ================= L0 =================
    h0 = acts.tile([128, 65, 513], BF16, tag="slotA")
    nc.vector.memset(h0[:, 0:1, :], 0.0)
    nc.vector.memset(h0[:, :, 0:1], 0.0)
    with tc.tile_pool(name="lp_l0", bufs=1) as hrp, \
         tc.tile_pool(name="pp_l0", bufs=4, space="PSUM") as psp:
        w_l0 = wload('w_l0', hrp)
        xs = hrp.tile([96, 65, 518], BF16, tag="xs")
        nc.vector.memset(xs[:, 64:65, :], 0.0)
        nc.vector.memset(xs[:, :, 514:518], 0.0)
        xsr = xs.rearrange("(c rp s) g x -> s c rp g x", rp=8, s=4)
        for s in range(4):
            for c in range(3):
                nc.sync.dma_start(
                    out=xsr[s][c, :, 0:64, 0:518 - s],
                    in_=din['x_pad'][c, 0:512, s:518].rearrange(
                        "(g rp) x -> rp g x", rp=8))
            nc.sync.dma_start(
                out=xsr[s][:, 0:6, 64, 0:518 - s],
                in_=din['x_pad'][:, 512:518, s:518])
        for gi in range(64):
            ps = psp.tile([128, 512], F32, tag="ps")
            k = 0
            for dg in range(2):
                for kxp in range(2):
                    nc.tensor.matmul(ps, lhsT=w_l0[:, dg, kxp, :],
                                     rhs=xs[:, gi + dg, 4 * kxp:4 * kxp + 512],
                                     start=(k == 0), stop=(k == 3))
                    k += 1
            nc.scalar.copy(out=h0[:, 1 + gi, 1:513], in_=ps)
    _inorm_relu(nc, sm, pp_s, h0[:, 1:65, 1:513],
                [h0[:, 1 + i, 1:513] for i in range(64)], ones8)
    if stage_done('h0', h0):
        return din

    # ================= D0 (direct-AP) =================
    d0 = acts.tile([128, 65, 257], BF16, tag="slotB")
    nc.vector.memset(d0[:, 0:1, :], 0.0)
    nc.vector.memset(d0[:, :, 0:1], 0.0)
    with tc.tile_pool(name="lp_d0", bufs=1) as hrp, \
         tc.tile_pool(name="pp_d0", bufs=4, space="PSUM") as psp:
        w_d0 = wload('w_d0', hrp)
        for G in range(0, 64, 2):
            ps = psp.tile([128, 2, 256], F32, tag="ps")
            for kx in range(3):
                nc.tensor.matmul(ps, lhsT=w_d0[:, 0, kx, :],
                                 rhs=h0[:, 1 + G:3 + G, kx:kx + 511:2],
                                 start=(kx == 0), stop=False)
            for kx in range(3):
                nc.tensor.matmul(ps, lhsT=w_d0[:, 1, kx, :],
                                 rhs=h0[:, G:G + 2, kx:kx + 511:2],
                                 start=False, stop=(kx == 2))
            nc.scalar.copy(out=d0[:, 1 + G:3 + G, 1:257], in_=ps)
    _inorm_relu(nc, sm, pp_s, d0[:, 1:65, 1:257],
                [d0[:, 1 + i, 1:257] for i in range(64)], ones4)
    if stage_done('d0', d0):
        return din

    # ================= D1 (direct-AP) =================
    d1 = acts.tile([128, 65, 129], BF16, tag="slotA")
    nc.vector.memset(d1[:, 0:1, :], 0.0)
    nc.vector.memset(d1[:, :, 0:1], 0.0)
    with tc.tile_pool(name="lp_d1", bufs=1) as hrp, \
         tc.tile_pool(name="pp_d1", bufs=4, space="PSUM") as psp:
        w_d1 = wload('w_d1', hrp)
        for G in range(0, 64, 4):
            ps = psp.tile([128, 4, 128], F32, tag="ps")
            for kx in range(3):
                nc.tensor.matmul(ps, lhsT=w_d1[:, 0, kx, :],
                                 rhs=d0[:, 1 + G:5 + G, kx:kx + 255:2],
                                 start=(kx == 0), stop=False)
            for kx in range(3):
                nc.tensor.matmul(ps, lhsT=w_d1[:, 1, kx, :],
                                 rhs=d0[:, G:G + 4, kx:kx + 255:2],
                                 start=False, stop=(kx == 2))
            nc.scalar.copy(out=d1[:, 1 + G:5 + G, 1:129], in_=ps)
    _inorm_relu(nc, sm, pp_s, d1[:, 1:65, 1:129],
                [d1[:, 1 + i, 1:129] for i in range(64)], ones2)
    if stage_done('d1', d1):
        return din

    # ================= D2 (direct-AP) =================
    d2 = acts.tile([128, 65, 65], BF16, tag="slotB")
    nc.vector.memset(d2[:, 0:1, :], 0.0)
    nc.vector.memset(d2[:, :, 0:1], 0.0)
    with tc.tile_pool(name="lp_d2", bufs=1) as hrp, \
         tc.tile_pool(name="pp_d2", bufs=4, space="PSUM") as psp:
        w_d2 = wload('w_d2', hrp)
        for Y0 in range(0, 64, 8):
            ps = psp.tile([128, 8, 64], F32, tag="ps")
            for kx in range(3):
                nc.tensor.matmul(ps, lhsT=w_d2[:, 0, kx, :],
                                 rhs=d1[:, 1 + Y0:9 + Y0, kx:kx + 127:2],
                                 start=(kx == 0), stop=False)
            for kx in range(3):
                nc.tensor.matmul(ps, lhsT=w_d2[:, 1, kx, :],
                                 rhs=d1[:, Y0:Y0 + 8, kx:kx + 127:2],
                                 start=False, stop=(kx == 2))
            nc.scalar.copy(out=d2[:, 1 + Y0:9 + Y0, 1:65], in_=ps)
    _inorm_relu(nc, sm, pp_s, d2[:, 1:65, 1:65],
                [d2[:, 1 + i, 1:65] for i in range(64)], None)
    if stage_done('d2', d2):
        return din

    # ================= D3 =================
    d3 = acts.tile([128, 2, 33, 33], BF16, tag="slotA")
    nc.vector.memset(d3[:, :, 32:33, :], 0.0)
    nc.vector.memset(d3[:, :, :, 32:33], 0.0)
    with tc.tile_pool(name="lp_d3", bufs=1) as hrp, \
         tc.tile_pool(name="pp_d3", bufs=4, space="PSUM") as psp:
        w_d3 = wload('w_d3', hrp)
        for h in range(2):
            for blk in range(2):
                ps = psp.tile([128, 16, 32], F32, tag="ps")
                first = True
                for ky in range(3):
                    for kx in range(3):
                        s0 = 32 * blk + ky
                        rhs = d2[:, s0:s0 + 31:2, kx:kx + 63:2]
                        nc.tensor.matmul(ps, lhsT=w_d3[:, h, ky * 3 + kx, :],
                                         rhs=rhs, start=first,
                                         stop=(ky == 2 and kx == 2))
                        first = False
                nc.scalar.copy(out=d3[:, h, 16 * blk:16 * blk + 16, 0:32], in_=ps)
    for h in range(2):
        _inorm_relu(nc, sm, pp_s, d3[:, h, 0:32, 0:32],
                    [d3[:, h, i, 0:32] for i in range(32)], None)
    if stage_done('d3', d3):
        return din

    # ================= U0 =================
    u0 = acts.tile([128, 65, 65], BF16, tag="slotB")
    nc.vector.memset(u0[:, 64:65, :], 0.0)
    nc.vector.memset(u0[:, :, 64:65], 0.0)
    with tc.tile_pool(name="lp_u0", bufs=1) as hrp, \
         tc.tile_pool(name="pp_u0", bufs=4, space="PSUM") as psp:
        w_u0 = wload('w_u0', hrp)
        for a in range(2):
            for b in range(2):
                for blk in range(2):
                    ps = psp.tile([128, 16, 32], F32, tag="ps")
                    mms = [(h, va, vb) for h in range(2) for va in range(2)
                           for vb in range(2)
                           if 0 <= a + 1 - 2 * va < 3 and 0 <= b + 1 - 2 * vb < 3]
                    for mi, (h, va, vb) in enumerate(mms):
                        rhs = d3[:, h, 16 * blk + va:16 * blk + va + 16,
                                 vb:vb + 32]
                        nc.tensor.matmul(ps, lhsT=w_u0[:, h, a, b, va, vb, :],
                                         rhs=rhs, start=(mi == 0),
                                         stop=(mi == len(mms) - 1))
                    nc.scalar.copy(
                        out=u0[:, 32 * blk + a:32 * blk + a + 31:2, b:b + 63:2],
                        in_=ps)
    _inorm_relu(nc, sm, pp_s, u0[:, 0:64, 0:64],
                [u0[:, i, 0:64] for i in range(64)], None)
    if stage_done('u0', u0):
        return din

    # ================= U1 =================
    u1 = acts.tile([128, 65, 65, 2], BF16, tag="slotA")
    nc.vector.memset(u1[:, 64:65, :, :], 0.0)
    nc.vector.memset(u1[:, :, 64:65, :], 0.0)
    with tc.tile_pool(name="lp_u1", bufs=1) as hrp, \
         tc.tile_pool(name="pp_u1", bufs=4, space="PSUM") as psp:
        w_u1 = wload('w_u1', hrp)
        for b in range(2):
            for blk in range(8):
                ps = psp.tile([128, 8, 64], F32, tag="ps")
                mms = [(t, hoff) for t in range(2) for hoff in range(2)]
                for mi, (t, hoff) in enumerate(mms):
                    rhs = u0[:, 8 * blk + t:8 * blk + t + 8, hoff:hoff + 64]
                    nc.tensor.matmul(ps, lhsT=w_u1[:, b, t, hoff, :], rhs=rhs,
                                     start=(mi == 0), stop=(mi == len(mms) - 1))
                nc.scalar.copy(out=u1[:, 8 * blk:8 * blk + 8, 0:64, b], in_=ps)
    u1x = u1.rearrange("p g x b -> p g (x b)")
    _inorm_relu(nc, sm, pp_s, u1x[:, 0:64, 0:128],
                [u1x[:, i, 0:128] for i in range(64)], ones2)
    if stage_done('u1', u1):
        return din

    # ================= U2 (direct-AP) =================
    u2 = acts.tile([128, 65, 258], BF16, tag="slotB")
    nc.vector.memset(u2[:, 64:65, :], 0.0)
    nc.vector.memset(u2[:, :, 0:1], 0.0)
    nc.vector.memset(u2[:, :, 257:258], 0.0)
    u1x = u1.rearrange("p g x b -> p g (x b)")
    with tc.tile_pool(name="lp_u2", bufs=1) as hrp, \
         tc.tile_pool(name="pp_u2", bufs=4, space="PSUM") as psp:
        w_u2 = wload('w_u2', hrp)
        for G in range(0, 64, 4):
            pse = psp.tile([128, 4, 128], F32, tag="ps")
            nc.tensor.matmul(pse, lhsT=w_u2[:, 0, :],
                             rhs=u1x[:, G:G + 4, 0:128], start=True, stop=False)
            nc.tensor.matmul(pse, lhsT=w_u2[:, 1, :],
                             rhs=u1x[:, G + 1:G + 5, 0:128], start=False,
                             stop=True)
            nc.scalar.copy(out=u2[:, G:G + 4, 1:257:2], in_=pse)
            pso = psp.tile([128, 4, 128], F32, tag="ps")
            nc.tensor.matmul(pso, lhsT=w_u2[:, 2, :],
                             rhs=u1x[:, G:G + 4, 1:129], start=True, stop=False)
            nc.tensor.matmul(pso, lhsT=w_u2[:, 3, :],
                             rhs=u1x[:, G:G + 4, 0:128], start=False,
                             stop=False)
            nc.tensor.matmul(pso, lhsT=w_u2[:, 4, :],
                             rhs=u1x[:, G + 1:G + 5, 1:129], start=False,
                             stop=False)
            nc.tensor.matmul(pso, lhsT=w_u2[:, 5, :],
                             rhs=u1x[:, G + 1:G + 5, 0:128], start=False,
                             stop=True)
            nc.scalar.copy(out=u2[:, G:G + 4, 2:258:2], in_=pso)
    _inorm_relu(nc, sm, pp_s, u2[:, 0:64, 1:257],
                [u2[:, i, 1:257] for i in range(64)], ones4)
    if stage_done('u2', u2):
        return din

    # ================= U3 (direct-AP) =================
    u3 = acts.tile([128, 64, 520], BF16, tag="slotA")
    with tc.tile_pool(name="lp_u3", bufs=1) as hrp, \
         tc.tile_pool(name="pp_u3", bufs=4, space="PSUM") as psp:
        w_u3 = wload('w_u3', hrp)
        for G in range(0, 64, 2):
            pse = psp.tile([128, 2, 256], F32, tag="ps")
            nc.tensor.matmul(pse, lhsT=w_u3[:, 0, :],
                             rhs=u2[:, G:G + 2, 1:257], start=True, stop=False)
            nc.tensor.matmul(pse, lhsT=w_u3[:, 1, :],
                             rhs=u2[:, G + 1:G + 3, 1:257], start=False,
                             stop=True)
            nc.scalar.copy(out=u3[:, G:G + 2, 4:516:2], in_=pse)
            pso = psp.tile([128, 2, 256], F32, tag="ps")
            nc.tensor.matmul(pso, lhsT=w_u3[:, 2, :],
                             rhs=u2[:, G:G + 2, 2:258], start=True, stop=False)
            nc.tensor.matmul(pso, lhsT=w_u3[:, 3, :],
                             rhs=u2[:, G:G + 2, 1:257], start=False, stop=False)
            nc.tensor.matmul(pso, lhsT=w_u3[:, 4, :],
                             rhs=u2[:, G + 1:G + 3, 2:258], start=False,
                             stop=False)
            nc.tensor.matmul(pso, lhsT=w_u3[:, 5, :],
                             rhs=u2[:, G + 1:G + 3, 1:257], start=False,
                             stop=True)
            nc.scalar.copy(out=u3[:, G:G + 2, 5:517:2], in_=pso)
    _inorm_relu(nc, sm, pp_s, u3[:, :, 4:516],
                [u3[:, i, 4:516] for i in range(64)], ones8)
    for dst, src in [(3, 5), (2, 6), (1, 7), (516, 514), (517, 513), (518, 512)]:
        nc.scalar.copy(out=u3[:, :, dst:dst + 1], in_=u3[:, :, src:src + 1])
    if stage_done('u3', u3):
        return din

    # ================= L_out (direct-AP; M=(dy*4+q), q=3 rows -> ones) =====
    f_nat = acts.tile([128, 16, 512], BF16, tag="slotB")
    with tc.tile_pool(name="lp_lo", bufs=1) as hrp, \
         tc.tile_pool(name="pp_lo", bufs=6, space="PSUM") as psp:
        w_lo = wload('w_lo', hrp)
        w_lo0 = wload('w_lo0', hrp)
        w_lo15 = wload('w_lo15', hrp)
        b_lo = wload('b_lo', hrp)
        for grp in range(16):
            ps = psp.tile([128, 512], F32, tag="ps")
            taps = []
            for e in range(6):
                gp = 4 * grp + e - 1
                if gp < 0 or gp > 63:
                    continue
                for kx in range(7):
                    if grp == 0 and e == 1:
                        lh = w_lo0[:, kx, :]
                    elif grp == 15 and e == 4:
                        lh = w_lo15[:, kx, :]
                    else:
                        lh = w_lo[:, e, kx, :]
                    taps.append((lh, gp, kx))
            for i, (lh, gp, kx) in enumerate(taps):
                nc.tensor.matmul(ps, lhsT=lh, rhs=u3[:, gp, kx + 1:kx + 513],
                                 start=(i == 0), stop=(i == len(taps) - 1))
            nc.scalar.activation(f_nat[:, grp, :], ps, AF.Tanh, bias=b_lo,
                                 scale=1.0)
    if stage_done('f', f_nat):
        return din

    # ================= segment mean =================
    segp = ctx.enter_context(tc.tile_pool(name="segbig", bufs=1))
    ident128 = wload('ident128', segp)
    iota_oh = wload('iota_oh', segp)
    iota32 = wload('iota32', segp)

    # ids natural layout (independent of f; scheduled early)
    ids_nat = segp.tile([128, 4, 512], BF16)  # [ylow, rb, x]
    nc.sync.dma_start(out=ids_nat,
                      in_=din['ids'].rearrange("(rb y x) -> y rb x", y=128,
                                               x=512))

    sums_sb = sm.tile([4, 32], F32, tag="sums_sb")
    nc.vector.memset(sums_sb, 0.0)
    idsT_t = segp.tile([128, 4, 4, 128], BF16)
    with tc.tile_pool(name="pp_tr", bufs=2, space="PSUM") as ppt, \
         tc.tile_pool(name="pp_sums", bufs=2, space="PSUM") as pps, \
         tc.tile_pool(name="segoh", bufs=6) as ohp, \
         tc.tile_pool(name="segft", bufs=6) as ftp:
        # idsT_t[p, xb, rb, ylow] = ids[Y=128*rb+ylow, 128*xb+p]
        for rb in range(4):
            for xb in range(4):
                pt = ppt.tile([128, 128], BF16)
                nc.tensor.transpose(pt,
                                    ids_nat[:, rb, 128 * xb:128 * xb + 128],
                                    ident128)
                nc.scalar.copy(out=idsT_t[:, xb, rb, :], in_=pt)

        # stage A: chunk (grp, xb, dy) = pixels (Y=32*grp+dy, x in xb-block):
        # psum[4,32] += fT[:, 4*dy:4*dy+4]^T @ oh[dy]
        idsv = idsT_t.rearrange("p xb rb y -> p xb (rb y)")  # [128, 4, 512]
        for j in range(64):
            grp, xb = j // 4, j % 4
            ptr = ppt.tile([128, 128], BF16)
            nc.tensor.transpose(ptr, f_nat[:, grp, 128 * xb:128 * (xb + 1)],
                                ident128)
            fT = ftp.tile([128, 128], BF16)
            nc.scalar.copy(out=fT, in_=ptr)
            oh = ohp.tile([128, 32, 32], BF16)
            ids_sl = idsv[:, xb, 32 * grp:32 * grp + 32]  # [128, 32] over dy
            nc.vector.tensor_tensor(
                out=oh,
                in0=ids_sl.unsqueeze(2).broadcast_to([128, 32, 32]),
                in1=iota_oh.unsqueeze(1).broadcast_to([128, 32, 32]),
                op=AL.is_equal)
            ps = pps.tile([4, 32], F32)
            for dy in range(32):
                nc.tensor.matmul(ps, lhsT=fT[:, 4 * dy:4 * dy + 4],
                                 rhs=oh[:, dy, :],
                                 start=(dy == 0), stop=(dy == 31))
            nc.vector.tensor_add(sums_sb, sums_sb, ps)

    if 'sums' in dbg:
        nc.sync.dma_start(out=dbg['sums'][:], in_=sums_sb)
    sums32 = sm.tile([32, 32], F32, tag="sums32")
    nc.vector.memset(sums32, 0.0)
    nc.vector.tensor_copy(out=sums32[0:4, :], in_=sums_sb)
    sumsT = sm.tile([32, 32], F32, tag="sumsT")
    nc.vector.transpose(sumsT, sums32)
    cntm = sm.tile([32, 1], F32, tag="cntm")
    nc.vector.tensor_scalar_max(cntm, sumsT[:, 3:4], 1.0)
    rcnt = sm.tile([32, 1], F32, tag="rcnt")
    nc.vector.reciprocal(rcnt, cntm)
    means_bf = sm.tile([32, 3], BF16, tag="means_bf")
    nc.vector.tensor_scalar_mul(means_bf, sumsT[:, 0:3], rcnt)
    if 'means' in dbg:
        nc.sync.dma_start(out=dbg['means'][:], in_=means_bf)
    bd = sm.tile([128, 12], BF16, tag="bd")
    nc.vector.memset(bd, 0.0)
    for s in range(4):
        nc.sync.dma_start(out=bd[32 * s:32 * s + 32, 3 * s:3 * s + 3],
                          in_=means_bf)

    # stage B: out[c, p] = means[c, id[p]] via block-diag one-hot matmul
    ids_q = din['ids'].rearrange("(q n) -> q n", q=4)
    out_r = out_d.rearrange("c (q x) -> q c x", q=4)
    with tc.tile_pool(name="seggat", bufs=6) as gat, \
         tc.tile_pool(name="segstg", bufs=3) as stgp, \
         tc.tile_pool(name="psumg", bufs=6, space="PSUM") as ppg:
        for t in range(32):
            ids_rep = gat.tile([128, 2048], BF16)
            for q in range(4):
                nc.sync.dma_start(
                    out=ids_rep[32 * q:32 * q + 32, :],
                    in_=ids_q[q:q + 1, t * 2048:(t + 1) * 2048].broadcast_to(
                        [32, 2048]))
            oh_g = gat.tile([128, 2048], BF16)
            nc.vector.tensor_scalar(out=oh_g, in0=ids_rep, scalar1=iota32,
                                    scalar2=None, op0=AL.is_equal)
            stg = stgp.tile([12, 2048], F32)
            for w in range(4):
                psg = ppg.tile([12, 512], F32)
                nc.tensor.matmul(psg, lhsT=bd,
                                 rhs=oh_g[:, w * 512:(w + 1) * 512],
                                 start=True, stop=True)
                if w % 2 == 0:
                    nc.scalar.copy(out=stg[:, 512 * w:512 * (w + 1)], in_=psg)
                else:
                    nc.vector.tensor_copy(out=stg[:, 512 * w:512 * (w + 1)],
                                          in_=psg)
            for q in range(4):
                nc.scalar.dma_start(
                    out=out_r[q, :, 2048 * t:2048 * (t + 1)],
                    in_=stg[3 * q:3 * q + 3, :])
    return din


# ======================================================================
# public entry: kernel(**inputs) with FULL batch inputs, 8-core SPMD
# ======================================================================
import concourse.bacc as _bacc
from concourse import bass_utils as _bass_utils

_CACHE = {}


def _get_nc():
    if 'nc' not in _CACHE:
        nc = _bacc.Bacc("TRN2", target_bir_lowering=False)
        with contextlib.ExitStack() as ctx:
            tc = ctx.enter_context(tile.TileContext(nc, pool_alloc_mode="queue"))
            build(nc, tc, ctx, upto='seg')
        nc.compile()
        _CACHE['nc'] = nc
    return _CACHE['nc']


def kernel(**inputs):
    nc = _get_nc()
    x = np.asarray(inputs['x'])
    ids = np.asarray(inputs['instance_map'])
    B = x.shape[0]
    shared = None
    in_maps = []
    for bi in range(B):
        inp0 = {k: v for k, v in inputs.items()}
        inp0['x'] = x[bi]
        inp0['instance_map'] = ids[bi]
        if shared is None:
            m = pack_inputs(inp0)
            shared = {k: v for k, v in m.items() if k not in ('x_pad', 'ids')}
        else:
            m = dict(shared)
            xp = np.pad(np.asarray(inp0['x'], np.float32), ((0, 0), (3, 3), (3, 3)),
                        mode='reflect')
            m['x_pad'] = _bf(xp)
            m['ids'] = _bf(np.asarray(inp0['instance_map']).reshape(-1).astype(
                np.float32))
        in_maps.append(m)
    res = _bass_utils.run_bass_kernel_spmd(nc, in_maps, core_ids=list(range(B)))
    out = np.stack([res.results[i]['out'].reshape(3, 512, 512) for i in range(B)])
    return out.astype(np.float32)


def kernel_traced(**inputs):
    """Like kernel() but with NTFF tracing; returns (out, exec_time_ns, profile)."""
    nc = _get_nc()
    x = np.asarray(inputs['x'])
    ids = np.asarray(inputs['instance_map'])
    B = x.shape[0]
    shared = None
    in_maps = []
    for bi in range(B):
        inp0 = {k: v for k, v in inputs.items()}
        inp0['x'] = x[bi]
        inp0['instance_map'] = ids[bi]
        if shared is None:
            m = pack_inputs(inp0)
            shared = {k: v for k, v in m.items() if k not in ('x_pad', 'ids')}
        else:
            m = dict(shared)
            xp = np.pad(np.asarray(inp0['x'], np.float32), ((0, 0), (3, 3), (3, 3)),
                        mode='reflect')
            m['x_pad'] = _bf(xp)
            m['ids'] = _bf(np.asarray(inp0['instance_map']).reshape(-1).astype(
                np.float32))
        in_maps.append(m)
    res = _bass_utils.run_bass_kernel_spmd(nc, in_maps, core_ids=list(range(B)),
                                           trace=True)
    out = np.stack([res.results[i]['out'].reshape(3, 512, 512) for i in range(B)])
    return out.astype(np.float32), res.exec_time_ns, res

